# revision 1
# baseline (speedup 1.0000x reference)
"""DGCNN-style EdgeConv layer + per-point MLP on 8 Trainium2 NeuronCores.

Strategy (data-parallel over batch, 2 batches per core):
  kernel1 (per core, 2 batches):
    - scores s_ij = dot(p_i,p_j) - |p_i|^2/2 - |p_j|^2/2 = -d_ij/2 via one
      K=5 PE matmul per 128-row block (correction rows baked into operands)
    - exact top-5 (incl self) per row via DVE max8 + max_index (fp32,
      first-occurrence ties == jax.lax.top_k tie order)
    - neighbor gather via gpsimd ap_gather
    - conv1 (3->64, edge = nbr - center folded into a K=6 matmul with [W;-W])
    - running max over k (gpsimd), running sum h / h^2 (ACT accum + DVE)
  host: combine per-core h moments -> global BN scale/bias (g=1>0 so
    max_k commutes with the monotone BN+LeakyReLU)
  kernel2 (per core): x1 = LeakyReLU(scale*max_k h + bias); 6-layer MLP on PE.
"""

import numpy as np

B, N, K = 16, 4096, 5
NCORES = 8
BPC = B // NCORES          # batches per core
PB = N // 128              # row blocks per batch (32)
NT = BPC * PB              # row blocks per core (64)
EPS = 1e-5
SLOPE = 0.2
HID = 64
COUNT = B * N * K          # BN sample count

_cache = {}


def _build_kernel1():
    import concourse.bass as bass
    import concourse.tile as tile
    from concourse import bacc, mybir
    from concourse.masks import make_identity
    from contextlib import ExitStack

    dt = mybir.dt
    AF = mybir.ActivationFunctionType
    ALU = mybir.AluOpType

    nc = bacc.Bacc("TRN2", target_bir_lowering=False, debug=False,
                   num_devices=NCORES)

    xs_ap = nc.dram_tensor("xs", [BPC, N, 6], dt.float32, kind="ExternalInput").ap()
    wc_ap = nc.dram_tensor("wc_pm", [6, 64], dt.float32, kind="ExternalInput").ap()
    x1_ap = nc.dram_tensor("x1", [64, BPC * N], dt.float32, kind="ExternalOutput").ap()
    hs_ap = nc.dram_tensor("hsums", [64, 2], dt.float32, kind="ExternalOutput").ap()
    idx_scr = nc.dram_tensor("idx_scr", [BPC, N, K], dt.uint16)  # internal bounce

    with tile.TileContext(nc) as tc, ExitStack() as ctx:
        glob = ctx.enter_context(tc.tile_pool(name="glob", bufs=1))
        # persistent tiles
        S_L = glob.tile([5, BPC * N], dt.float32)   # rows x,y,z,1,-sq/2
        S_R = glob.tile([5, BPC * N], dt.float32)   # rows x,y,z,-sq/2,1
        idxcol = glob.tile([128, NT * K], dt.uint16)
        hparts = glob.tile([64, 2 * K * 8], dt.float32)  # sum h | sum h^2 parts

        # ---- phase A: load x, build S_L / S_R via PE transposes ----
        with tc.tile_pool(name="pa", bufs=1) as pa, \
             tc.tile_pool(name="pa2", bufs=2) as pa2, \
             tc.tile_pool(name="psA", bufs=2, space="PSUM") as psA:
            xt = pa.tile([128, BPC * 32 * 6], dt.float32)
            # xs[b, c*128+p, d] -> xt[p, b*192 + c*6 + d]
            nc.sync.dma_start(
                xt[:], xs_ap.rearrange("b (c p) d -> p (b c d)", p=128))
            ident = pa.tile([128, 128], dt.float32)
            make_identity(nc, ident[:])
            CC = pa.tile([128, NT * 10], dt.float32)
            # coords into cols t*10+(0..2) and t*10+(5..7)
            src_xyz = xt[:].rearrange("p (t d) -> p t d", d=6)[:, :, 0:3]
            nc.vector.tensor_copy(
                CC[:].rearrange("p (t c) -> p t c", c=10)[:, :, 0:3], src_xyz)
            nc.vector.tensor_copy(
                CC[:].rearrange("p (t c) -> p t c", c=10)[:, :, 5:8], src_xyz)
            # sq sums
            sq3 = pa.tile([128, NT * 6], dt.float32)
            nc.vector.tensor_mul(sq3[:], xt[:], xt[:])
            sq3v = sq3[:].rearrange("p (t d) -> p t d", d=6)
            tmp = pa.tile([128, NT], dt.float32)
            nc.vector.tensor_add(tmp[:], sq3v[:, :, 0:1], sq3v[:, :, 1:2])
            nc.vector.tensor_add(tmp[:], tmp[:], sq3v[:, :, 2:3])
            ccv = CC[:].rearrange("p (t c) -> p t c", c=10)
            nc.vector.tensor_scalar_mul(ccv[:, :, 4:5], tmp[:], -0.5)
            nc.vector.tensor_copy(ccv[:, :, 8:9], ccv[:, :, 4:5])
            nc.vector.memset(ccv[:, :, 3:4], 1.0)
            nc.vector.memset(ccv[:, :, 9:10], 1.0)
            # transposes: CC[:, t*10:(t+1)*10] -> [10, 128] -> S_L/S_R cols
            for t in range(NT):
                pst = psA.tile([10, 128], dt.float32)
                nc.tensor.transpose(pst[:], CC[:, t * 10:(t + 1) * 10], ident[:])
                nc.scalar.activation(S_L[:, t * 128:(t + 1) * 128], pst[0:5, :],
                                     AF.Identity, scale=1.0)
                nc.scalar.activation(S_R[:, t * 128:(t + 1) * 128], pst[5:10, :],
                                     AF.Identity, scale=1.0)

        # ---- phase B: distance scores + exact top-5 ----
        with tc.tile_pool(name="pb", bufs=2) as pb, \
             tc.tile_pool(name="pbs", bufs=2) as pbs, \
             tc.tile_pool(name="psB", bufs=2, space="PSUM") as psB:
            for t in range(NT):
                b = t // PB
                lhsT = S_L[:, t * 128:(t + 1) * 128]
                sc = pb.tile([128, N], dt.float32, tag="sc")
                for h in range(2):
                    ps = psB.tile([128, 2048], dt.float32, tag="ps")
                    for s in range(4):
                        off = b * N + h * 2048 + s * 512
                        nc.tensor.matmul(ps[:, s * 512:(s + 1) * 512], lhsT,
                                         S_R[:, off:off + 512],
                                         start=True, stop=True)
                    nc.scalar.activation(sc[:, h * 2048:(h + 1) * 2048], ps[:],
                                         AF.Identity, scale=1.0)
                vals = pbs.tile([128, 8], dt.float32, tag="vals")
                idxs = pbs.tile([128, 8], dt.uint16, tag="idxs")
                nc.vector.max(vals[:], sc[:])
                nc.vector.max_index(idxs[:], vals[:], sc[:])
                nc.vector.tensor_copy(idxcol[:, t * K:(t + 1) * K], idxs[:, 0:K])
            # idxcol -> DRAM: idx_scr[b, rb*128+p, k] = idxcol[p, (b*PB+rb)*K + k]
            nc.sync.dma_start(
                idx_scr.ap().rearrange("b (rb p) k -> p (b rb k)", p=128),
                idxcol[:])

        # ---- phase C: gather + conv + maxpool + moments ----
        with tc.tile_pool(name="pc", bufs=1) as pc, \
             tc.tile_pool(name="pce", bufs=2) as pce, \
             tc.tile_pool(name="psC", bufs=4, space="PSUM") as psC:
            tabs = pc.tile([128, N], dt.float32)
            nc.vector.memset(tabs[:], 0.0)
            for q in range(8):
                cb = q // 4
                nc.sync.dma_start(tabs[16 * q:16 * q + 3, :],
                                  S_L[0:3, cb * N:(cb + 1) * N])
            idx16 = pc.tile([128, 320], dt.int16)
            # load per-core interleaved index lists
            # core q, batch cb=q//4, quarter qq=q%4: j = k*1024 + 16*nh + nl
            # placed at [16q+nl, k*64+nh]; src = idx_scr[cb, qq*1024+16*nh+nl, k]
            for q in range(8):
                cb, qq = q // 4, q % 4
                src = idx_scr.ap()[cb, qq * 1024:(qq + 1) * 1024, :] \
                    .rearrange("(nh nl) k -> nl (k nh)", nl=16)
                nc.sync.dma_start(idx16[16 * q:16 * (q + 1), :],
                                  src.bitcast(dt.int16))
            gout = pc.tile([128, 5120], dt.float32)
            nc.gpsimd.ap_gather(gout[:], tabs[:], idx16[:], channels=128,
                                num_elems=N, d=1, num_idxs=5120)
            Wc = pc.tile([6, 64], dt.float32)
            nc.sync.dma_start(Wc[:], wc_ap[:])
            for q in range(8):
                cb, qq = q // 4, q % 4
                edge = pce.tile([6, 5120], dt.float32, tag="edge")
                nc.sync.dma_start(edge[0:3, :], gout[16 * q:16 * q + 3, :])
                cbase = cb * N + qq * 1024
                for k in range(K):
                    nc.sync.dma_start(edge[3:6, k * 1024:(k + 1) * 1024],
                                      S_L[0:3, cbase:cbase + 1024])
                x1q = pce.tile([64, 1024], dt.float32, tag="x1q")
                for k in range(K):
                    t = q * K + k
                    hps = psC.tile([64, 1024], dt.float32, tag="hps")
                    for s in range(2):
                        c0 = k * 1024 + s * 512
                        nc.tensor.matmul(hps[:, s * 512:(s + 1) * 512], Wc[:],
                                         edge[:, c0:c0 + 512],
                                         start=True, stop=True)
                    hk = pce.tile([64, 1024], dt.float32, tag="hk")
                    nc.scalar.activation(hk[:], hps[:], AF.Identity, scale=1.0,
                                         accum_out=hparts[:, t:t + 1])
                    # sum h^2 (exact, on DVE): out = (hk*1)*hk, accum add
                    sqs = pce.tile([64, 1024], dt.float32, tag="sqs")
                    nc.vector.scalar_tensor_tensor(
                        sqs[:], hk[:], 1.0, hk[:], ALU.mult, ALU.mult,
                        accum_out=hparts[:, 40 + t:41 + t])
                    if k == 0:
                        nc.gpsimd.tensor_copy(x1q[:], hk[:])
                    else:
                        nc.gpsimd.tensor_max(x1q[:], x1q[:], hk[:])
                nc.sync.dma_start(
                    x1_ap[:, cb * N + qq * 1024: cb * N + (qq + 1) * 1024],
                    x1q[:])
            hsums = pc.tile([64, 2], dt.float32)
            nc.vector.tensor_reduce(hsums[:, 0:1], hparts[:, 0:40],
                                    mybir.AxisListType.X, ALU.add)
            nc.vector.tensor_reduce(hsums[:, 1:2], hparts[:, 40:80],
                                    mybir.AxisListType.X, ALU.add)
            nc.sync.dma_start(hs_ap[:], hsums[:])

    nc.finalize()
    return nc


def _build_kernel2():
    import concourse.bass as bass
    import concourse.tile as tile
    from concourse import bacc, mybir
    from contextlib import ExitStack

    dt = mybir.dt
    ALU = mybir.AluOpType
    AF = mybir.ActivationFunctionType
    M = BPC * N  # points per core (8192)

    nc = bacc.Bacc("TRN2", target_bir_lowering=False, debug=False,
                   num_devices=NCORES)

    x1_ap = nc.dram_tensor("x1", [64, M], dt.float32, kind="ExternalInput").ap()
    sb_ap = nc.dram_tensor("scale_bias", [64, 2], dt.float32, kind="ExternalInput").ap()
    w1_ap = nc.dram_tensor("w1", [64, HID], dt.float32, kind="ExternalInput").ap()
    w2_ap = nc.dram_tensor("w2", [HID, 128], dt.float32, kind="ExternalInput").ap()
    w3_ap = nc.dram_tensor("w3", [128, 256], dt.float32, kind="ExternalInput").ap()
    w4_ap = nc.dram_tensor("w4", [256, 128], dt.float32, kind="ExternalInput").ap()
    w5_ap = nc.dram_tensor("w5", [128, HID], dt.float32, kind="ExternalInput").ap()
    w6b_ap = nc.dram_tensor("w6b", [HID + 1, 13], dt.float32, kind="ExternalInput").ap()
    b15_ap = nc.dram_tensor("b15", [256, 5], dt.float32, kind="ExternalInput").ap()
    out_ap = nc.dram_tensor("out", [BPC, N, 13], dt.float32, kind="ExternalOutput").ap()

    NCH = M // 512   # 16 chunks of 512 for layers 1-5
    with tile.TileContext(nc) as tc, ExitStack() as ctx:
        cpool = ctx.enter_context(tc.tile_pool(name="c", bufs=1))
        acts = ctx.enter_context(tc.tile_pool(name="acts", bufs=6))
        psum = ctx.enter_context(tc.tile_pool(name="ps", bufs=8, space="PSUM"))

        w1 = cpool.tile([64, HID], dt.float32); nc.sync.dma_start(w1[:], w1_ap[:])
        w2 = cpool.tile([HID, 128], dt.float32); nc.sync.dma_start(w2[:], w2_ap[:])
        w3a = cpool.tile([128, 128], dt.float32); nc.sync.dma_start(w3a[:], w3_ap[:, 0:128])
        w3b = cpool.tile([128, 128], dt.float32); nc.sync.dma_start(w3b[:], w3_ap[:, 128:256])
        w4a = cpool.tile([128, 128], dt.float32); nc.sync.dma_start(w4a[:], w4_ap[0:128, :])
        w4b = cpool.tile([128, 128], dt.float32); nc.sync.dma_start(w4b[:], w4_ap[128:256, :])
        w5 = cpool.tile([128, HID], dt.float32); nc.sync.dma_start(w5[:], w5_ap[:])
        w6b = cpool.tile([HID + 1, 13], dt.float32); nc.sync.dma_start(w6b[:], w6b_ap[:])
        b15 = cpool.tile([256, 5], dt.float32); nc.sync.dma_start(b15[:], b15_ap[:])
        sb = cpool.tile([64, 2], dt.float32); nc.sync.dma_start(sb[:], sb_ap[:])

        x1 = acts.tile([64, M], dt.float32, tag="a")
        nc.sync.dma_start(x1[:], x1_ap[:])
        # y = scale*x + bias ; z = max(y, 0.2*y)
        y = acts.tile([64, M], dt.float32, tag="b")
        nc.vector.tensor_scalar(y[:], x1[:], sb[:, 0:1], sb[:, 1:2],
                                ALU.mult, ALU.add)
        h0 = acts.tile([65, M], dt.float32, tag="c")
        nc.vector.scalar_tensor_tensor(h0[0:64, :], y[:], SLOPE, y[:],
                                       ALU.mult, ALU.max)

        def layer(dst, dst_rows, lhsTs, rhs_list, bias_col, bias_rows, nch=NCH):
            # dst[:, chunk] = relu(sum_i lhsTs[i].T @ rhs_list[i][:, chunk] + b)
            csz = M // nch
            nmm = csz // 512
            for c in range(nch):
                ps = psum.tile([dst_rows, csz], dt.float32, tag="mm")
                for s in range(nmm):
                    sl = slice(c * csz + s * 512, c * csz + (s + 1) * 512)
                    for i, (lh, rh) in enumerate(zip(lhsTs, rhs_list)):
                        nc.tensor.matmul(ps[:, s * 512:(s + 1) * 512], lh,
                                         rh[:, sl], start=(i == 0),
                                         stop=(i == len(lhsTs) - 1))
                nc.vector.tensor_scalar(
                    dst[:, c * csz:(c + 1) * csz], ps[:],
                    b15[bias_rows, bias_col:bias_col + 1], 0.0,
                    ALU.add, ALU.max)

        h1 = acts.tile([64, M], dt.float32, tag="a")
        layer(h1[:], 64, [w1[:]], [h0[0:64, :]], 0, slice(0, 64))
        h2 = acts.tile([128, M], dt.float32, tag="b")
        layer(h2[:], 128, [w2[:]], [h1[:]], 1, slice(0, 128))
        h3a = acts.tile([128, M], dt.float32, tag="d")
        layer(h3a[:], 128, [w3a[:]], [h2[:]], 2, slice(0, 128))
        h3b = acts.tile([128, M], dt.float32, tag="e")
        layer(h3b[:], 128, [w3b[:]], [h2[:]], 2, slice(128, 256))
        h4 = acts.tile([128, M], dt.float32, tag="a")
        layer(h4[:], 128, [w4a[:], w4b[:]], [h3a[:], h3b[:]], 3, slice(0, 128))
        h5 = acts.tile([65, M], dt.float32, tag="c")
        layer(h5[0:64, :], 64, [w5[:]], [h4[:]], 4, slice(0, 64))
        nc.vector.memset(h5[64:65, :], 1.0)

        outsb = cpool.tile([128, 64 * 13], dt.float32)
        for c in range(M // 128):
            ps = psum.tile([128, 13], dt.float32, tag="fin")
            nc.tensor.matmul(ps[:], h5[:, c * 128:(c + 1) * 128], w6b[:],
                             start=True, stop=True)
            nc.scalar.activation(outsb[:, c * 13:(c + 1) * 13], ps[:],
                                 AF.Identity, scale=1.0)
        # outsb[p, c*13+j] -> out[b, (c*128+p) mod-ish, j]; c = b*32 + cc
        nc.sync.dma_start(
            out_ap.rearrange("b (c p) j -> p (b c j)", p=128), outsb[:])

    nc.finalize()
    return nc


def _get_programs():
    if "k1" not in _cache:
        _cache["k1"] = _build_kernel1()
        _cache["k2"] = _build_kernel2()
    return _cache["k1"], _cache["k2"]


def kernel(x, conv_w, bn_g, bn_b, w1, b1, w2, b2, w3, b3, w4, b4, w5, b5,
           w6, b6):
    from concourse.bass_utils import run_bass_kernel_spmd

    k1, k2 = _get_programs()
    x = np.asarray(x, dtype=np.float32)
    wc_pm = np.concatenate([np.asarray(conv_w), -np.asarray(conv_w)], axis=0) \
        .astype(np.float32)

    in1 = [{"xs": np.ascontiguousarray(x[BPC * c:BPC * (c + 1)]),
            "wc_pm": wc_pm} for c in range(NCORES)]
    r1 = run_bass_kernel_spmd(k1, in1, list(range(NCORES))).results

    # host: combine BN moments -> scale/bias
    tot = np.zeros((64, 2), np.float64)
    for c in range(NCORES):
        tot += r1[c]["hsums"].astype(np.float64)
    mean = tot[:, 0] / COUNT
    var = tot[:, 1] / COUNT - mean ** 2
    scale = (np.asarray(bn_g, np.float64) / np.sqrt(var + EPS))
    bias = np.asarray(bn_b, np.float64) - mean * scale
    sb = np.stack([scale, bias], axis=1).astype(np.float32)

    b15 = np.zeros((256, 5), np.float32)
    b15[0:64, 0] = b1; b15[0:128, 1] = b2; b15[0:256, 2] = b3
    b15[0:128, 3] = b4; b15[0:64, 4] = b5
    w6b = np.concatenate([np.asarray(w6), np.asarray(b6)[None, :]], axis=0) \
        .astype(np.float32)

    in2 = [{"x1": r1[c]["x1"], "scale_bias": sb,
            "w1": np.asarray(w1, np.float32), "w2": np.asarray(w2, np.float32),
            "w3": np.asarray(w3, np.float32), "w4": np.asarray(w4, np.float32),
            "w5": np.asarray(w5, np.float32), "w6b": w6b, "b15": b15}
           for c in range(NCORES)]
    r2 = run_bass_kernel_spmd(k2, in2, list(range(NCORES))).results

    out = np.concatenate([r2[c]["out"] for c in range(NCORES)], axis=0)
    return out
